# revision 13
# baseline (speedup 1.0000x reference)
"""Trainium2 Bass kernel for nn_MultiHeadAttention_60541859004769.

Strategy: pure data-parallel over batch B=16 across 8 NeuronCores (2 batches
per core). Each core runs the full fused pipeline for its 2 batches with all
activations kept TRANSPOSED ([d, rows]) so every GEMM has its contraction on
partitions without extra transposes:

  vpT  = relu(v @ W_lv)^T               [d, rows]
  spT  = relu(s @ W_ls)^T               [d, rows]
  qT   = (vp @ W_q)^T                   [h*dk, rows]
  vsT  = (sp @ W_v)^T                   [h*dv, rows]
  resT = vpT + pos^T                    (spilled to DRAM scratch)
  per (b, h):  scoresT = vsT_h^T-contract qT_h   [lk, lq]  (K=64, head pairs
               expT    = exp(scoresT/8)                     pack PE row groups)
               ctxT_aug = [vs | 1]^T-contract expT [65, lq] (row 64 = sums)
               ctxT_h  = ctxT_aug[0:64] * recip(sums)
  outT = (ctx @ W_p)^T + b_p + resT
  out  = LayerNorm(transpose(outT))  -> natural layout, DMA to DRAM

GEMMs run as float32r (full-rate fp32); PE transposes run exact fp32.
Three 32KB/partition SBUF "slots" are time-shared: slotA = vpT -> vsT -> outT,
slotB = spT -> ctxT, slotC = qT. No collectives; host concatenates outputs.
"""

import sys
import types

import numpy as np

# --- NTFF profile hook shim (antenv.axon_hooks is absent in this image).
def _install_ntff_shim():
    if "antenv.axon_hooks" in sys.modules:
        return
    try:
        import antenv
    except ImportError:
        return
    mod = types.ModuleType("antenv.axon_hooks")
    _hook = [None]
    mod.set_axon_ntff_profile_hook = lambda h: _hook.__setitem__(0, h)
    mod.get_axon_ntff_profile_hook = lambda: _hook[0]
    sys.modules["antenv.axon_hooks"] = mod
    antenv.axon_hooks = mod
    try:
        from trn_agent_boot.trn_boot import _ntff_profile_via_ctypes
        mod.set_axon_ntff_profile_hook(
            _ntff_profile_via_ctypes("/opt/axon/libaxon_pjrt.so")
        )
    except Exception:
        pass


_install_ntff_shim()


def _axon_reset():
    """Reset the remote NeuronCores (clears a wedged device state)."""
    try:
        import ctypes
        import jax
        jax.devices()
        lib = ctypes.CDLL("/opt/axon/libaxon_pjrt.so")
        lib.axon_reset.restype = ctypes.c_int64
        lib.axon_reset()
    except Exception:
        pass


import concourse.bass as bass
import concourse.mybir as mybir
import concourse.tile as tile
from concourse import bacc
from concourse.bass_utils import run_bass_kernel_spmd
from concourse.masks import make_identity

F32 = mybir.dt.float32
F32R = mybir.dt.float32r
AF = mybir.ActivationFunctionType
ALU = mybir.AluOpType
AX = mybir.AxisListType

N_CORES = 8
B, LV, LS = 16, 512, 512
FEA_V, FEA_S = 2048, 768
D, H, DK, DV = 1024, 16, 64, 64
BL = B // N_CORES          # batches per core = 2
R = BL * LV                # rows per core = 1024
KV = FEA_V // 128          # 16
KS = FEA_S // 128          # 6
MD = D // 128              # 8 d-tiles
NCH = R // 512             # 2 row chunks of 512
EPS = 1e-6


def _r(ap):
    return ap.bitcast(F32R)


def build_nc():
    nc = bacc.Bacc("TRN2", target_bir_lowering=False, debug=False)

    dp = lambda name, shape: nc.declare_dram_parameter(name, shape, F32, isOutput=False)
    dpr = lambda name, shape: nc.declare_dram_parameter(name, shape, F32R, isOutput=False)
    v_d = dp("v", [R, FEA_V])
    s_d = dp("s", [R, FEA_S])
    wlv_d = dpr("W_lv", [FEA_V, D])
    blv_d = dp("b_lv", [D])
    wls_d = dpr("W_ls", [FEA_S, D])
    bls_d = dp("b_ls", [D])
    wq_d = dpr("W_q", [D, H * DK])
    bq_d = dp("b_q", [H * DK])
    wv_d = dpr("W_v", [D, H * DV])
    bv_d = dp("b_v", [H * DV])
    wp_d = dpr("W_p", [H * DV, D])
    bp_d = dp("b_p", [D])
    pos_d = dp("pos_v", [LV, D])
    gam_d = dp("ln_gamma", [D])
    bet_d = dp("ln_beta", [D])
    out_d = nc.declare_dram_parameter("out", [R, D], F32, isOutput=True)

    # DRAM scratch for the residual (vpT + posT), spilled to save SBUF
    res_scratch = nc.dram_tensor("res_scratch", [R, D], F32)

    from contextlib import ExitStack

    with tile.TileContext(nc) as tc, ExitStack() as top:
        singles = top.enter_context(tc.tile_pool(name="singles", bufs=1))

        ident = singles.tile([128, 128], F32, tag="ident", name="ident")
        make_identity(nc, ident[:])

        # stacked 64x64 identities so base-64 transposes have a matching-base
        # identity operand
        ident2 = singles.tile([128, 64], F32, tag="ident2", name="ident2")
        nc.gpsimd.memset(ident2[:], 0.0)
        for half in range(2):
            nc.gpsimd.affine_select(
                out=ident2[half * 64 : (half + 1) * 64, :],
                in_=ident2[half * 64 : (half + 1) * 64, :],
                compare_op=ALU.not_equal,
                fill=1.0,
                base=0,
                pattern=[[-1, 64]],
                channel_multiplier=1,
            )

        eps_t = singles.tile([128, 1], F32, tag="eps", name="eps")
        nc.gpsimd.memset(eps_t[:], EPS)
        ones_t = singles.tile([128, 1], F32, tag="ones", name="ones")
        nc.gpsimd.memset(ones_t[:], 1.0)

        # biases as [128, MD] (column m holds the bias slice for d-tile m)
        def load_bias(dram, name):
            t = singles.tile([128, MD], F32, tag=name, name=name)
            nc.sync.dma_start(t[:], dram.ap().rearrange("(m p) -> p m", p=128))
            return t

        blv_t = load_bias(blv_d, "blv")
        bls_t = load_bias(bls_d, "bls")
        bq_t = load_bias(bq_d, "bq")
        bv_t = load_bias(bv_d, "bv")
        bp_t = load_bias(bp_d, "bp")

        bp_b = singles.tile([128, D], F32, tag="bpb", name="bpb")
        nc.sync.dma_start(
            bp_b[:], bp_d.ap().rearrange("(o d) -> o d", o=1).partition_broadcast(128)
        )
        gam_t = singles.tile([128, D], F32, tag="gam", name="gam")
        nc.sync.dma_start(
            gam_t[:], gam_d.ap().rearrange("(o d) -> o d", o=1).partition_broadcast(128)
        )
        bet_t = singles.tile([128, D], F32, tag="bet", name="bet")
        nc.sync.dma_start(
            bet_t[:], bet_d.ap().rearrange("(o d) -> o d", o=1).partition_broadcast(128)
        )

        def alloc_list(pool, prefix, n, shape, dt=F32):
            return [
                pool.tile(shape, dt, tag=f"{prefix}{i}", name=f"{prefix}{i}")
                for i in range(n)
            ]

        # --- three time-shared 32KB/partition slots of 8 x [128, R] tiles ---
        big_pool = top.enter_context(tc.tile_pool(name="big", bufs=1))
        slotA = alloc_list(big_pool, "slotA_", MD, [128, R])
        slotB = alloc_list(big_pool, "slotB_", MD, [128, R])
        slotC = alloc_list(big_pool, "slotC_", MD, [128, R])

        # ========== Phase A: vpT = relu(v @ W_lv)^T  (into slotA) ==========
        vpT = slotA
        with ExitStack() as ph:
            vsl_pool = ph.enter_context(tc.tile_pool(name="vsl", bufs=4))
            t_psum = ph.enter_context(tc.tile_pool(name="t_psum", bufs=3, space="PSUM"))
            vTn_pool = ph.enter_context(tc.tile_pool(name="vTn", bufs=1))
            vTn = alloc_list(vTn_pool, "vTn", KV, [128, 512])
            wlv_pool = ph.enter_context(tc.tile_pool(name="wlv", bufs=4))
            a_psum = ph.enter_context(tc.tile_pool(name="a_psum", bufs=1, space="PSUM"))

            for n in range(NCH):
                # transpose this row-chunk of v: vTn[k] = v[chunk, k-slice]^T
                for k in range(KV):
                    for rt in range(4):
                        row0 = (n * 4 + rt) * 128
                        vsl = vsl_pool.tile([128, 128], F32, name="vsl")
                        nc.sync.dma_start(
                            vsl[:], v_d[row0 : row0 + 128, k * 128 : (k + 1) * 128]
                        )
                        ps = t_psum.tile([128, 128], F32, name="tps")
                        nc.tensor.transpose(ps[:], vsl[:], ident[:])
                        nc.vector.tensor_copy(
                            _r(vTn[k][:, rt * 128 : (rt + 1) * 128]), ps[:]
                        )
                for mh in range(2):
                    pss = [
                        a_psum.tile([128, 512], F32, tag=f"aps{m}", name=f"aps{m}")
                        for m in range(4)
                    ]
                    for k in range(KV):
                        wl = wlv_pool.tile([128, 512], F32R, name="wl")
                        nc.sync.dma_start(
                            wl[:],
                            wlv_d[k * 128 : (k + 1) * 128, mh * 512 : (mh + 1) * 512],
                        )
                        for mi in range(4):
                            nc.tensor.matmul(
                                pss[mi][:],
                                _r(wl[:, mi * 128 : (mi + 1) * 128]),
                                _r(vTn[k][:]),
                                start=(k == 0),
                                stop=(k == KV - 1),
                            )
                    for mi in range(4):
                        m = mh * 4 + mi
                        nc.scalar.activation(
                            _r(vpT[m][:, n * 512 : (n + 1) * 512]),
                            pss[mi][:],
                            AF.Relu,
                            bias=blv_t[:, m : m + 1],
                        )

        # ========== Phase B: spT = relu(s @ W_ls)^T  (into slotB) ==========
        spT = slotB
        with ExitStack() as ph:
            ssl_pool = ph.enter_context(tc.tile_pool(name="ssl", bufs=4))
            t2_psum = ph.enter_context(tc.tile_pool(name="t2_psum", bufs=3, space="PSUM"))
            sTn_pool = ph.enter_context(tc.tile_pool(name="sTn", bufs=1))
            sTn = alloc_list(sTn_pool, "sTn", KS, [128, 512])
            wls_pool = ph.enter_context(tc.tile_pool(name="wls", bufs=1))
            wls = alloc_list(wls_pool, "wls", KS, [128, D], dt=F32R)
            b_psum = ph.enter_context(tc.tile_pool(name="b_psum", bufs=1, space="PSUM"))

            for k in range(KS):
                nc.sync.dma_start(wls[k][:], wls_d[k * 128 : (k + 1) * 128, :])
            for n in range(NCH):
                for k in range(KS):
                    for rt in range(4):
                        row0 = (n * 4 + rt) * 128
                        ssl = ssl_pool.tile([128, 128], F32, name="ssl")
                        nc.sync.dma_start(
                            ssl[:], s_d[row0 : row0 + 128, k * 128 : (k + 1) * 128]
                        )
                        ps = t2_psum.tile([128, 128], F32, name="t2ps")
                        nc.tensor.transpose(ps[:], ssl[:], ident[:])
                        nc.vector.tensor_copy(
                            _r(sTn[k][:, rt * 128 : (rt + 1) * 128]), ps[:]
                        )
                for mh in range(2):
                    pss = [
                        b_psum.tile([128, 512], F32, tag=f"bps{m}", name=f"bps{m}")
                        for m in range(4)
                    ]
                    for k in range(KS):
                        for mi in range(4):
                            m = mh * 4 + mi
                            nc.tensor.matmul(
                                pss[mi][:],
                                _r(wls[k][:, m * 128 : (m + 1) * 128]),
                                _r(sTn[k][:]),
                                start=(k == 0),
                                stop=(k == KS - 1),
                            )
                    for mi in range(4):
                        m = mh * 4 + mi
                        nc.scalar.activation(
                            _r(spT[m][:, n * 512 : (n + 1) * 512]),
                            pss[mi][:],
                            AF.Relu,
                            bias=bls_t[:, m : m + 1],
                        )

        # ========== Phase C: qT = (vp @ W_q)^T + b_q  (into slotC) ==========
        qT = slotC
        with ExitStack() as ph:
            wq_pool = ph.enter_context(tc.tile_pool(name="wq", bufs=1))
            wq = alloc_list(wq_pool, "wq", MD, [128, D], dt=F32R)
            c_psum = ph.enter_context(tc.tile_pool(name="c_psum", bufs=1, space="PSUM"))
            for k in range(MD):
                nc.sync.dma_start(wq[k][:], wq_d[k * 128 : (k + 1) * 128, :])
            for mh in range(2):
                for n in range(NCH):
                    pss = [
                        c_psum.tile([128, 512], F32, tag=f"cps{m}", name=f"cps{m}")
                        for m in range(4)
                    ]
                    for k in range(MD):
                        for mi in range(4):
                            m = mh * 4 + mi
                            nc.tensor.matmul(
                                pss[mi][:],
                                _r(wq[k][:, m * 128 : (m + 1) * 128]),
                                _r(vpT[k][:, n * 512 : (n + 1) * 512]),
                                start=(k == 0),
                                stop=(k == MD - 1),
                            )
                    for mi in range(4):
                        m = mh * 4 + mi
                        nc.vector.tensor_scalar_add(
                            _r(qT[m][:, n * 512 : (n + 1) * 512]),
                            pss[mi][:],
                            bq_t[:, m : m + 1],
                        )

        # == Phase C2: res = vp + pos + b_p in NATURAL layout -> DRAM scratch ==
        with ExitStack() as ph:
            p_psum = ph.enter_context(tc.tile_pool(name="p_psum", bufs=3, space="PSUM"))
            rstage_pool = ph.enter_context(tc.tile_pool(name="rstage", bufs=2))
            posnat_pool = ph.enter_context(tc.tile_pool(name="posnat", bufs=2))

            for r in range(R // 128):
                rst = rstage_pool.tile([128, D], F32, name="rst")
                for m in range(MD):
                    ps = p_psum.tile([128, 128], F32, name="pps")
                    nc.tensor.transpose(
                        ps[:], vpT[m][:, r * 128 : (r + 1) * 128], ident[:]
                    )
                    nc.vector.tensor_copy(rst[:, m * 128 : (m + 1) * 128], ps[:])
                pn = posnat_pool.tile([128, D], F32, name="pn")
                prow = (r % (LV // 128)) * 128    # pos broadcast over batch
                nc.sync.dma_start(pn[:], pos_d[prow : prow + 128, :])
                nc.vector.tensor_add(rst[:], rst[:], pn[:])
                nc.vector.tensor_add(rst[:], rst[:], bp_b[:])
                nc.sync.dma_start(res_scratch[r * 128 : (r + 1) * 128, :], rst[:])

        # ========== Phase D: vsT = (sp @ W_v)^T + b_v  (into slotA) ==========
        vsT = slotA
        with ExitStack() as ph:
            wv_pool = ph.enter_context(tc.tile_pool(name="wv", bufs=1))
            wv = alloc_list(wv_pool, "wv", MD, [128, D], dt=F32R)
            d_psum = ph.enter_context(tc.tile_pool(name="d_psum", bufs=1, space="PSUM"))
            for k in range(MD):
                nc.sync.dma_start(wv[k][:], wv_d[k * 128 : (k + 1) * 128, :])
            for mh in range(2):
                for n in range(NCH):
                    pss = [
                        d_psum.tile([128, 512], F32, tag=f"dps{m}", name=f"dps{m}")
                        for m in range(4)
                    ]
                    for k in range(MD):
                        for mi in range(4):
                            m = mh * 4 + mi
                            nc.tensor.matmul(
                                pss[mi][:],
                                _r(wv[k][:, m * 128 : (m + 1) * 128]),
                                _r(spT[k][:, n * 512 : (n + 1) * 512]),
                                start=(k == 0),
                                stop=(k == MD - 1),
                            )
                    for mi in range(4):
                        m = mh * 4 + mi
                        nc.vector.tensor_scalar_add(
                            _r(vsT[m][:, n * 512 : (n + 1) * 512]),
                            pss[mi][:],
                            bv_t[:, m : m + 1],
                        )

        # ========== Phase E: attention per (batch, head pair) (into slotB) ===
        ctxT = slotB
        with ExitStack() as ph:
            vsaug_pool = ph.enter_context(tc.tile_pool(name="vsaug", bufs=8))
            exp_pool = ph.enter_context(tc.tile_pool(name="expT", bufs=4))
            recip_pool = ph.enter_context(tc.tile_pool(name="recip", bufs=2))
            recipb_pool = ph.enter_context(tc.tile_pool(name="recipb", bufs=2))
            vs_psum = ph.enter_context(tc.tile_pool(name="vs_psum", bufs=2, space="PSUM"))
            s_psum = ph.enter_context(tc.tile_pool(name="s_psum", bufs=2, space="PSUM"))
            ctx_psum = ph.enter_context(tc.tile_pool(name="ctx_psum", bufs=2, space="PSUM"))

            for b in range(BL):
                for hp in range(H // 2):   # head pairs share a 128-partition tile
                    mt = hp
                    heads = (2 * hp, 2 * hp + 1)
                    cols = slice(b * LV, (b + 1) * LV)

                    # vs natural layout (+ ones col) via PE transposes
                    vs_aug = {}
                    for h_i, h in enumerate(heads):
                        base = (h % 2) * 64
                        for j in range(LV // 128):
                            ps = vs_psum.tile([128, 64], F32, tag="vps", name="vps")
                            nc.tensor.transpose(
                                ps[:],
                                vsT[mt][base : base + 64,
                                        b * LV + j * 128 : b * LV + (j + 1) * 128],
                                ident2[base : base + 64, :],
                            )
                            aug = vsaug_pool.tile([128, 65], F32, tag="vsaug", name="vsaug")
                            nc.vector.tensor_copy(_r(aug[:, 0:64]), ps[:])
                            nc.vector.tensor_copy(_r(aug[:, 64:65]), ones_t[:])
                            vs_aug[(h_i, j)] = aug

                    # scoresT + exp; heads interleaved so their K=64 matmuls
                    # land in disjoint PE row groups and run concurrently
                    expts = {}
                    for jj in range(2):
                        pss = [
                            s_psum.tile([128, 1024], F32, tag="sps", name="sps")
                            for _ in range(2)
                        ]
                        for j2 in range(2):
                            j = jj * 2 + j2
                            for h_i, h in enumerate(heads):
                                base = (h % 2) * 64
                                nc.tensor.matmul(
                                    pss[h_i][:, j2 * 512 : (j2 + 1) * 512],
                                    _r(vsT[mt][base : base + 64,
                                               b * LV + j * 128 : b * LV + (j + 1) * 128]),
                                    _r(qT[mt][base : base + 64, cols]),
                                    start=True,
                                    stop=True,
                                )
                        for h_i in range(2):
                            et = exp_pool.tile([128, 1024], F32, tag="expT", name="expT")
                            nc.scalar.activation(
                                _r(et[:]), pss[h_i][:], AF.Exp, scale=0.125
                            )
                            expts[(h_i, jj)] = et

                    # ctxT_aug = [vs|1]^T-contract expT  (row 64 = softmax sums)
                    for h_i, h in enumerate(heads):
                        base = (h % 2) * 64
                        psc = ctx_psum.tile([65, 512], F32, tag="cxps", name="cxps")
                        for j in range(LV // 128):
                            nc.tensor.matmul(
                                psc[:],
                                _r(vs_aug[(h_i, j)][:]),
                                _r(expts[(h_i, j // 2)][:, (j % 2) * 512 : (j % 2 + 1) * 512]),
                                start=(j == 0),
                                stop=(j == 3),
                            )
                        rec = recip_pool.tile([1, 512], F32, tag="recip", name="recip")
                        nc.vector.reciprocal(rec[:], psc[64:65, :])
                        recb = recipb_pool.tile([64, 512], F32, tag="recipb", name="recipb")
                        nc.gpsimd.partition_broadcast(recb[:], rec[:])
                        nc.vector.tensor_tensor(
                            _r(ctxT[mt][base : base + 64, cols]),
                            psc[0:64, :],
                            recb[:],
                            op=ALU.mult,
                        )

        # ==== Phase F: outT = (ctx @ W_p)^T + b_p + resT  (into slotA) ======
        outT = slotA
        with ExitStack() as ph:
            wp_pool = ph.enter_context(tc.tile_pool(name="wp", bufs=1))
            wp = alloc_list(wp_pool, "wp", MD, [128, D], dt=F32R)
            f_psum = ph.enter_context(tc.tile_pool(name="f_psum", bufs=1, space="PSUM"))
            for k in range(MD):
                nc.sync.dma_start(wp[k][:], wp_d[k * 128 : (k + 1) * 128, :])
            for mh in range(2):
                for n in range(NCH):
                    pss = [
                        f_psum.tile([128, 512], F32, tag=f"fps{m}", name=f"fps{m}")
                        for m in range(4)
                    ]
                    for k in range(MD):
                        for mi in range(4):
                            m = mh * 4 + mi
                            nc.tensor.matmul(
                                pss[mi][:],
                                _r(wp[k][:, m * 128 : (m + 1) * 128]),
                                _r(ctxT[k][:, n * 512 : (n + 1) * 512]),
                                start=(k == 0),
                                stop=(k == MD - 1),
                            )
                    for mi in range(4):
                        m = mh * 4 + mi
                        sl = slice(n * 512, (n + 1) * 512)
                        nc.vector.tensor_copy(_r(outT[m][:, sl]), pss[mi][:])

        # ====== Phase G: transpose back, LayerNorm over d, DMA out ==========
        with ExitStack() as ph:
            g_psum = ph.enter_context(tc.tile_pool(name="g_psum", bufs=4, space="PSUM"))
            nat_pool = ph.enter_context(tc.tile_pool(name="nat", bufs=2))
            stat_pool = ph.enter_context(tc.tile_pool(name="stats", bufs=8))

            for r in range(R // 128):
                nat = nat_pool.tile([128, D], F32, tag="nat", name="nat")
                for m in range(MD):
                    ps = g_psum.tile([128, 128], F32, name="gps")
                    nc.tensor.transpose(
                        ps[:], outT[m][:, r * 128 : (r + 1) * 128], ident[:]
                    )
                    nc.vector.tensor_copy(nat[:, m * 128 : (m + 1) * 128], ps[:])

                rldn = nat_pool.tile([128, D], F32, tag="rldn", name="rldn")
                nc.sync.dma_start(rldn[:], res_scratch[r * 128 : (r + 1) * 128, :])
                nc.vector.tensor_add(nat[:], nat[:], rldn[:])

                ssum = stat_pool.tile([128, 1], F32, tag="ssum", name="ssum")
                nc.vector.tensor_reduce(ssum[:], nat[:], axis=AX.X, op=ALU.add)
                ssq = stat_pool.tile([128, 1], F32, tag="ssq", name="ssq")
                scr = nat_pool.tile([128, D], F32, tag="scr", name="scr", bufs=1)
                nc.vector.tensor_tensor(scr[:], nat[:], nat[:], op=ALU.mult)
                nc.vector.tensor_reduce(ssq[:], scr[:], axis=AX.X, op=ALU.add)
                nmean = stat_pool.tile([128, 1], F32, tag="nmean", name="nmean")
                nc.vector.tensor_scalar_mul(nmean[:], ssum[:], -1.0 / D)
                ex2 = stat_pool.tile([128, 1], F32, tag="ex2", name="ex2")
                nc.vector.tensor_scalar_mul(ex2[:], ssq[:], 1.0 / D)
                msq = stat_pool.tile([128, 1], F32, tag="msq", name="msq")
                nc.vector.tensor_tensor(msq[:], nmean[:], nmean[:], op=ALU.mult)
                var = stat_pool.tile([128, 1], F32, tag="var", name="var")
                nc.vector.tensor_tensor(var[:], ex2[:], msq[:], op=ALU.subtract)
                std = stat_pool.tile([128, 1], F32, tag="std", name="std")
                nc.scalar.activation(std[:], var[:], AF.Sqrt, bias=eps_t[:])
                rstd = stat_pool.tile([128, 1], F32, tag="rstd", name="rstd")
                nc.vector.reciprocal(rstd[:], std[:])
                nmr = stat_pool.tile([128, 1], F32, tag="nmr", name="nmr")
                nc.vector.tensor_tensor(nmr[:], nmean[:], rstd[:], op=ALU.mult)

                xhat = nat_pool.tile([128, D], F32, tag="xhat", name="xhat")
                nc.scalar.activation(
                    xhat[:], nat[:], AF.Identity, bias=nmr[:], scale=rstd[:]
                )
                yv = nat_pool.tile([128, D], F32, tag="yv", name="yv")
                nc.vector.tensor_tensor(yv[:], xhat[:], gam_t[:], op=ALU.mult)
                nc.vector.tensor_add(yv[:], yv[:], bet_t[:])
                nc.sync.dma_start(out_d[r * 128 : (r + 1) * 128, :], yv[:])

    nc.compile()
    return nc


_NC_CACHE = None


def get_nc():
    global _NC_CACHE
    if _NC_CACHE is None:
        _NC_CACHE = build_nc()
    return _NC_CACHE


def make_in_maps(inputs):
    shared = {}
    for name in ("W_lv", "b_lv", "W_ls", "b_ls", "W_q", "b_q", "W_v", "b_v",
                 "W_p", "b_p", "ln_gamma", "ln_beta"):
        shared[name] = np.ascontiguousarray(np.asarray(inputs[name], dtype=np.float32))
    shared["pos_v"] = np.ascontiguousarray(
        np.asarray(inputs["pos_v"], dtype=np.float32).reshape(LV, D)
    )
    v = np.asarray(inputs["v"], dtype=np.float32)
    s = np.asarray(inputs["s"], dtype=np.float32)
    in_maps = []
    for c in range(N_CORES):
        m = dict(shared)
        m["v"] = np.ascontiguousarray(v[c * BL : (c + 1) * BL].reshape(R, FEA_V))
        m["s"] = np.ascontiguousarray(s[c * BL : (c + 1) * BL].reshape(R, FEA_S))
        in_maps.append(m)
    return in_maps


def _run(nc, in_maps, trace=False):
    try:
        return run_bass_kernel_spmd(
            nc, in_maps, core_ids=list(range(N_CORES)), trace=trace
        )
    except Exception:
        _axon_reset()
        return run_bass_kernel_spmd(
            nc, in_maps, core_ids=list(range(N_CORES)), trace=trace
        )


def kernel(**inputs) -> np.ndarray:
    nc = get_nc()
    in_maps = make_in_maps(inputs)
    res = _run(nc, in_maps)
    out = np.concatenate(
        [res.results[c]["out"].reshape(BL, LV, D) for c in range(N_CORES)], axis=0
    )
    return out


def run_traced(inputs):
    """For test.py: run with NTFF tracing; returns (output, BassKernelResults)."""
    nc = get_nc()
    in_maps = make_in_maps(inputs)
    res = _run(nc, in_maps, trace=True)
    out = np.concatenate(
        [res.results[c]["out"].reshape(BL, LV, D) for c in range(N_CORES)], axis=0
    )
    return out, res
